# revision 1
# baseline (speedup 1.0000x reference)
"""Trainium2 Bass kernel for nn_DistMultDecoder:

    score[e] = sum_h uh[e,h] * vh[e,h] * w_relation[etypes[e], h]

for E=1,000,000 edges, H=128 hidden, R=64 relations, fp32.

Strategy (memory-bound problem, ~1.03 GB of HBM traffic):
  * Edges are sharded data-parallel across the 8 NeuronCores (125k each,
    padded to 123*1024 = 125952 so every DMA tile uses all 128 SBUF
    partitions - 16-port DMA striping needs 128 partitions; 125-partition
    tiles measured 3x lower HBM bandwidth).
  * Per [128 x 8-edge] tile, the relation rows rel[e,:] are materialized
    on-chip as one-hot(etypes) @ w_relation on the TensorEngine (bf16, with
    w_relation split into bf16 hi+lo parts accumulated in fp32 PSUM, which
    keeps fp32-grade accuracy at bf16 matmul speed), so the only HBM traffic
    is uh, vh (fp32), etypes (bf16) and the scores - no per-edge gather.
  * one-hot masks: etypes replicated to 64 partitions by a DRAM-broadcast
    read, compared against a per-partition iota on the VectorEngine.
  * uh*vh runs on GPSIMD, (uh*vh)*rel and the hidden-dim reduction on the
    VectorEngine, DMA dispatch split between the SP and Activation queues.
  * Scores are written in a partition-major device layout and unpermuted on
    the host.

The kernel function takes the FULL inputs and returns the FULL output.
"""
import numpy as np
import ml_dtypes

import concourse.bass as bass
import concourse.tile as tile
from concourse import bacc, mybir
import concourse.bass_utils as bass_utils

N_CORES = 8
E_TOTAL = 1_000_000
E_CORE = E_TOTAL // N_CORES   # 125000
H = 128
R = 64
P = 128                        # SBUF partitions per tile
F = 8                          # edges per partition per tile
E_TILE = P * F                 # 1024
N_TILES = 123
E_PAD = N_TILES * E_TILE       # 125952 padded edges per core
GROUPS = [8] * 15 + [3]        # tiles per etypes-batching group (sum 123)
F32 = mybir.dt.float32
BF16 = mybir.dt.bfloat16

_nc_cache = {}


def _build_nc(repeat=1, num_devices=N_CORES, io_bufs=6, mid_bufs=3, pp_bufs=2):
    key = (repeat, num_devices, io_bufs, mid_bufs, pp_bufs)
    if key in _nc_cache:
        return _nc_cache[key]
    nc = bacc.Bacc("TRN2", target_bir_lowering=False, debug=False,
                   num_devices=num_devices)
    uh = nc.dram_tensor("uh", [E_PAD, H], F32, kind="ExternalInput").ap()
    vh = nc.dram_tensor("vh", [E_PAD, H], F32, kind="ExternalInput").ap()
    et = nc.dram_tensor("et", [E_PAD], BF16, kind="ExternalInput").ap()
    wrh = nc.dram_tensor("wrh", [R, H], BF16, kind="ExternalInput").ap()
    wrl = nc.dram_tensor("wrl", [R, H], BF16, kind="ExternalInput").ap()
    sc = nc.dram_tensor("sc", [E_PAD], F32, kind="ExternalOutput").ap()

    uh_r = uh.rearrange("(m p f) h -> m p (f h)", p=P, f=F)   # m = tile index
    vh_r = vh.rearrange("(m p f) h -> m p (f h)", p=P, f=F)

    with tile.TileContext(nc) as tc:
        with (tc.tile_pool(name="io", bufs=io_bufs) as io,
              tc.tile_pool(name="grp", bufs=2) as grp,
              tc.tile_pool(name="mid", bufs=mid_bufs) as mid,
              tc.tile_pool(name="pp", bufs=pp_bufs, space="PSUM") as pp,
              tc.tile_pool(name="const", bufs=1) as const):
            wrh_sb = const.tile([R, H], BF16)
            nc.sync.dma_start(out=wrh_sb, in_=wrh)
            wrl_sb = const.tile([R, H], BF16)
            nc.sync.dma_start(out=wrl_sb, in_=wrl)
            riota = const.tile([R, 1], F32)
            nc.gpsimd.iota(riota, pattern=[[0, 1]], base=0, channel_multiplier=1,
                           allow_small_or_imprecise_dtypes=True)

            def grp_body(tile0, g):
                e_grp = g * E_TILE
                ebase = tile0 * E_TILE
                # etypes replicated to 64 partitions straight from DRAM
                et_b = grp.tile([R, e_grp], BF16, tag="etb")
                src = et[ebase:ebase + e_grp]
                et_src = bass.AP(tensor=src.tensor, offset=src.offset,
                                 ap=[[0, R]] + src.ap)
                nc.scalar.dma_start(out=et_b, in_=et_src)
                mask = grp.tile([R, e_grp], BF16, tag="mask")
                nc.vector.tensor_scalar(
                    out=mask, in0=et_b, scalar1=riota, scalar2=None,
                    op0=mybir.AluOpType.is_equal)
                # mask column j = t*(P*F) + pp*F + ff for tile t in group
                mask_r = mask.rearrange("r (t p f) -> r t p f", t=g, f=F)
                sc_g = grp.tile([P, F * g], F32, tag="sc")

                for t in range(g):
                    m = tile0 + t
                    uh_t = io.tile([P, F * H], F32, tag="uh")
                    vh_t = io.tile([P, F * H], F32, tag="vh")
                    nc.sync.dma_start(out=uh_t, in_=uh_r[m])
                    nc.scalar.dma_start(out=vh_t, in_=vh_r[m])
                    rel = pp.tile([P, F * H], F32, tag="rel")
                    rel_r = rel.rearrange("p (f h) -> p f h", f=F)
                    for j in range(F):
                        nc.tensor.matmul(rel_r[:, j], mask_r[:, t, :, j], wrh_sb,
                                         start=True, stop=False)
                        nc.tensor.matmul(rel_r[:, j], mask_r[:, t, :, j], wrl_sb,
                                         start=False, stop=True)
                    p_t = mid.tile([P, F * H], F32, tag="p")
                    nc.gpsimd.tensor_mul(p_t, uh_t, vh_t)
                    t_t = mid.tile([P, F * H], F32, tag="t")
                    nc.vector.tensor_mul(t_t, p_t, rel)
                    t_r = t_t.rearrange("p (f h) -> p f h", f=F)
                    nc.vector.tensor_reduce(
                        sc_g[:, t * F:(t + 1) * F], t_r,
                        axis=mybir.AxisListType.X, op=mybir.AluOpType.add)
                sc_out = sc[ebase:ebase + e_grp].rearrange("(p w) -> p w", p=P)
                nc.scalar.dma_start(out=sc_out, in_=sc_g)

            def run_all():
                tile0 = 0
                for g in GROUPS:
                    grp_body(tile0, g)
                    tile0 += g

            if repeat == 1:
                run_all()
            else:
                with tc.For_i(0, repeat, 1):
                    run_all()

    nc.compile()
    _nc_cache[key] = nc
    return nc


def _prep_core_inputs(uh, vh, et, wrh, wrl):
    """Pad one core shard to E_PAD and assemble the input map."""
    pad = E_PAD - uh.shape[0]
    uh = np.ascontiguousarray(uh, dtype=np.float32)
    vh = np.ascontiguousarray(vh, dtype=np.float32)
    etb = np.asarray(et).astype(ml_dtypes.bfloat16)
    if pad:
        uh = np.concatenate([uh, np.zeros((pad, H), np.float32)])
        vh = np.concatenate([vh, np.zeros((pad, H), np.float32)])
        etb = np.concatenate([etb, np.zeros((pad,), ml_dtypes.bfloat16)])
    return {"uh": uh, "vh": vh, "et": etb, "wrh": wrh, "wrl": wrl}


def _unpermute_score(sc_flat):
    """Undo the device's partition-major score layout for one core."""
    out = np.empty(E_PAD, np.float32)
    tile0 = 0
    for g in GROUPS:
        base = tile0 * E_TILE
        blk = sc_flat[base:base + g * E_TILE].reshape(P, g, F)
        out[base:base + g * E_TILE] = blk.transpose(1, 0, 2).reshape(-1)
        tile0 += g
    return out[:E_CORE]


def kernel(uh, vh, etypes, w_relation):
    uh = np.asarray(uh)
    vh = np.asarray(vh)
    etypes = np.asarray(etypes)
    wr = np.asarray(w_relation, dtype=np.float32)
    assert uh.shape == (E_TOTAL, H) and vh.shape == (E_TOTAL, H)
    assert etypes.shape == (E_TOTAL,)

    # w_relation split into bf16 hi + lo; 1.0*hi + 1.0*lo accumulated in fp32
    # PSUM reproduces fp32 w_relation to ~2^-17 relative accuracy.
    wrh = wr.astype(ml_dtypes.bfloat16)
    wrl = (wr - wrh.astype(np.float32)).astype(ml_dtypes.bfloat16)

    nc = _build_nc(repeat=1, num_devices=N_CORES)
    in_maps = []
    for c in range(N_CORES):
        sl = slice(c * E_CORE, (c + 1) * E_CORE)
        in_maps.append(_prep_core_inputs(uh[sl], vh[sl], etypes[sl], wrh, wrl))

    res = bass_utils.run_bass_kernel_spmd(nc, in_maps,
                                          core_ids=list(range(N_CORES)))
    score = np.concatenate(
        [_unpermute_score(res.results[c]["sc"]) for c in range(N_CORES)])
    return score.astype(np.float32)


if __name__ == "__main__":
    rng = np.random.default_rng(0)
    uh = rng.standard_normal((E_TOTAL, H), dtype=np.float32)
    vh = rng.standard_normal((E_TOTAL, H), dtype=np.float32)
    et = rng.integers(0, R, size=E_TOTAL, dtype=np.int32)
    wr = ((rng.random((R, H), dtype=np.float32) - 0.5) * 0.625)
    out = kernel(uh, vh, et, wr)
    exp = np.einsum("eh,eh->e", uh * vh, wr[et]).astype(np.float32)
    rel = np.abs(out - exp).max() / np.abs(exp).max()
    print(f"self-test scale-relative error: {rel:.3e}")
